# revision 26
# baseline (speedup 1.0000x reference)
"""AttentionMILPooling Trainium2 kernel.

Math (matches the jax reference):
    scores  = tanh(X @ W1 + b1) @ W2 + b2          # [T, 1]
    weights = softmax(scores, axis=0)              # global over all T
    out[b]  = sum_{i in bag b} weights[i] * X[i]   # [64, 512]

Key identities used:
  * b2 cancels exactly in the softmax, so it is dropped.
  * scores are bounded (|s| <= sum|W2| ~ 13) so no max-subtraction is
    needed; exp stays well inside fp32 range.
  * out[b] = (sum_{i in b} exp(s_i) * X_i) / Z with Z = sum_i exp(s_i):
    each core computes unnormalized per-bag sums U and returns all its
    per-row exp values; the host sums Z globally and divides once.
  * All math is row-order-free within a bag, so rows are permuted by the
    DMA layout (each partition loads 2 consecutive DRAM rows -> 4KB
    descriptors) without any correction.

Sharding: instances split contiguously across 8 cores (16384 rows each);
with equal 2048-row bags every core owns 8 whole bags. The tiny MLP
weights are replicated. Each core returns U [8 x 512] and its exp
values w [128 x 128]; the host concatenates and divides by global Z.

Per-core device pipeline, per 512-row group (32 groups), all matmul
operands bf16 with fp32 PSUM accumulation:
  SWDGE : X fp32 -> SBUF bf16 (cast in DMA)             xb [128,4,512]
  PE    : 16x transpose -> X^T chunks in PSUM
  DVE   : 4x copy PSUM->SBUF                            xt [128,4,512]
  PE    : 8x matmul H^T = W1^T @ X^T (accum f chunks)   [128,512] f32
  ACT   : 2x tanh(H^T + b1) -> bf16 (b1 as per-partition bias)
  PE    : 8x matmul s = tanhH^T.T @ W2 (accum h chunks) [128,4] f32
  ACT   : 1x exp(s) -> persistent wsave columns (bf16)
  PE    : 4x matmul U[bag] += w_tile^T @ X_tile (w stationary, N=512)
Emission is software-pipelined (s for group g-1, U for g-2) so the
in-order PE stream never waits on same-group ACT results. Per-bag U
accumulates in a dedicated [1,512] PSUM bank over the bag's 16 tiles
(PSUM start=True zeroes a whole 2KB region, so each accumulation group
owns its bank), then is copied to SBUF and DMA'd out.
"""

import numpy as np

N_CORES = 8
T_FULL = 131072
F = 512  # feature dim
HID = 256  # hidden dim
B_FULL = 64  # number of bags
P = 128  # partitions

_COMPILED_CACHE = {}


def _build_program(n_tiles, tile_col, n_cols):
    """Build the SPMD bass program.

    n_tiles: number of 128-row tiles per core (must be divisible by 4).
    tile_col: list, local bag-column index for each tile (same on all cores).
    n_cols: number of local bag columns.
    """
    import concourse.bacc as bacc
    import concourse.mybir as mybir
    from concourse.tile import TileContext
    from concourse.masks import make_identity

    f32 = mybir.dt.float32
    bf16 = mybir.dt.bfloat16
    FC = F // P  # 4 feature chunks
    MC = HID // P  # 2 hidden chunks
    rows_per_core = n_tiles * P
    JT = 4  # 128-row subtiles per group
    GR = JT * P  # rows per group
    n_groups = n_tiles // JT

    nc = bacc.Bacc(
        "TRN2", target_bir_lowering=False, debug=False, num_devices=N_CORES
    )

    x = nc.declare_dram_parameter("x", [rows_per_core, F], f32, isOutput=False)
    w1 = nc.declare_dram_parameter("w1", [F, HID], f32, isOutput=False)
    b1 = nc.declare_dram_parameter("b1", [HID], f32, isOutput=False)
    w2 = nc.declare_dram_parameter("w2", [HID, 1], f32, isOutput=False)
    u_out = nc.declare_dram_parameter("u", [n_cols, F], f32, isOutput=True)
    w_out = nc.declare_dram_parameter("w", [P, n_tiles], f32, isOutput=True)

    with TileContext(nc) as tc:
        with (
            tc.tile_pool(name="const", bufs=1) as const_pool,
            tc.tile_pool(name="xb", bufs=8) as xb_pool,
            tc.tile_pool(name="xf", bufs=2) as xf_pool,
            tc.tile_pool(name="xt", bufs=4) as xt_pool,
            tc.tile_pool(name="th", bufs=3) as th_pool,
            tc.tile_pool(name="out_sb", bufs=1) as out_pool,
            tc.tile_pool(name="pt", bufs=4, space="PSUM") as pt_pool,
            tc.tile_pool(name="hp", bufs=2, space="PSUM") as hp_pool,
            tc.tile_pool(name="sp", bufs=1, space="PSUM") as sp_pool,
            tc.tile_pool(name="acc", bufs=1, space="PSUM") as acc_pool,
        ):
            # ---- prefetch the first X groups before the const DMAs so the
            # transposes (which need only xb + ident) start ASAP ----
            xb_hist = {}

            def emit_load(gg):
                # load + cast fp32 -> bf16 during DMA (SWDGE). Rows are
                # permuted so each partition reads 4 consecutive DRAM rows
                # (8KB contiguous, one descriptor run per partition):
                # subtile j holds rows {4p + j} of the 512-row group. All
                # downstream math is row-order-free within a bag (sums),
                # and every 512-row group lies in one bag (bag % 512 == 0).
                xb = xb_pool.tile([P, JT, F], bf16, name="xb", tag="xb")
                xb_hist[gg] = xb
                x_src = x[gg * GR : (gg + 1) * GR, :].rearrange(
                    "(p q) f -> p q f", p=P
                )
                nc.gpsimd.dma_start(out=xb, in_=x_src)

            def emit_load_hwdge(gg):
                # First groups go through HWDGE + DVE cast: SWDGE descriptor
                # generation sits behind ~6us of GpSimd preamble barriers at
                # kernel start, while the HWDGE RTL path is free immediately.
                # Two half-size DMAs per group so the earliest transposes
                # start sooner (rows {2p+q} per 256-row window - any
                # within-bag permutation is legal).
                xb = xb_pool.tile([P, JT, F], bf16, name="xb", tag="xb")
                xb_hist[gg] = xb
                for h in range(2):
                    sl = x[
                        gg * GR + h * (GR // 2) : gg * GR + (h + 1) * (GR // 2), :
                    ].rearrange("(p q) f -> p q f", p=P)
                    xf = xf_pool.tile([P, 2, F], f32, name="xf", tag="xf")
                    nc.sync.dma_start(out=xf, in_=sl)
                    nc.vector.tensor_copy(
                        out=xb[:, 2 * h : 2 * h + 2, :], in_=xf
                    )

            emit_load_hwdge(0)
            if n_groups > 1:
                emit_load_hwdge(1)

            # ---- constants ----
            ident = const_pool.tile([P, P], bf16)
            make_identity(nc, ident)

            # W1 -> bf16 chunks: w1b[p, c, m, j] = W1[c*128+p, m*128+j]
            w1f = const_pool.tile([P, FC, MC, P], f32)
            nc.sync.dma_start(
                out=w1f, in_=w1.rearrange("(c p) (m j) -> p c m j", p=P, j=P)
            )
            w1b = const_pool.tile([P, FC, MC, P], bf16)
            nc.vector.tensor_copy(out=w1b, in_=w1f)

            # W2 -> bf16 chunks: w2b[p, m, 0] = W2[m*128+p, 0]
            w2f = const_pool.tile([P, MC, 1], f32)
            nc.sync.dma_start(
                out=w2f, in_=w2.rearrange("(m p) one -> p m one", p=P)
            )
            w2b = const_pool.tile([P, MC, 1], bf16)
            nc.vector.tensor_copy(out=w2b, in_=w2f)

            # b1 per-partition bias: b1s[p, m] = b1[m*128+p]
            b1s = const_pool.tile([P, MC], f32)
            nc.sync.dma_start(out=b1s, in_=b1.rearrange("(m p) -> p m", p=P))

            # softmax weights for every row, bf16 (also read back by the host
            # to form the global softmax denominator)
            wsave = const_pool.tile([P, n_tiles], bf16)

            # ---- per-bag accumulator handling ----
            # matmul outputs must start at partition 0/32/64, so each bag
            # accumulates in its own [1, F] PSUM tile (one bank) for the
            # contiguous run of its tiles, then is copied into its row of
            # the SBUF result before the tile is recycled.
            u_sb = out_pool.tile([1, n_cols, F], f32)
            first_tile = {}
            last_tile = {}
            for t, cl in enumerate(tile_col):
                first_tile.setdefault(cl, t)
                last_tile[cl] = t
            u_bag = [None] * n_cols

            # ---- main loop over 512-row groups (software-pipelined) ----
            # PE executes in emission order, so same-group dependencies on
            # ACT (tanh -> s, exp -> U) would stall it. Emit s for group
            # g-1 and U for group g-2: their ACT inputs are long ready.
            th_hist = {}
            sp_hist = {}

            def emit_s(gg):
                th_g = th_hist[gg]
                sp = sp_pool.tile([P, JT], f32, name="sp", tag="sp")
                sp_hist[gg] = sp
                for j in range(JT):
                    for m in range(MC):
                        nc.tensor.matmul(
                            sp[:, j : j + 1],
                            th_g[:, m, j, :],
                            w2b[:, m, :],
                            start=(j == 0 and m == 0),
                            stop=(j == JT - 1 and m == MC - 1),
                        )
                nc.scalar.activation(
                    wsave[:, gg * JT : (gg + 1) * JT],
                    sp,
                    mybir.ActivationFunctionType.Exp,
                )
                del sp_hist[gg]
                del th_hist[gg]

            def emit_u(gg):
                xb_g = xb_hist.pop(gg)
                for j in range(JT):
                    t = JT * gg + j
                    col = tile_col[t]
                    if u_bag[col] is None:
                        u_bag[col] = acc_pool.tile(
                            [1, F], f32, name="u_bag", tag="u_bag"
                        )
                    nc.tensor.matmul(
                        u_bag[col],
                        wsave[:, t : t + 1],
                        xb_g[:, j, :],
                        start=(t == first_tile[col]),
                        stop=(t == last_tile[col]),
                    )
                    if t == last_tile[col]:
                        nc.vector.tensor_copy(
                            out=u_sb[:, col, :], in_=u_bag[col]
                        )
                        u_bag[col] = None

            for g in range(n_groups):
                if g not in xb_hist:
                    emit_load(g)
                xb = xb_hist[g]

                # transpose all subtiles/chunks: two PSUM tiles of 8 each
                pts = []
                for h in range(2):
                    pt = pt_pool.tile([P, 2, FC, P], bf16, tag="pt")
                    pts.append(pt)
                    for jj in range(2):
                        j = 2 * h + jj
                        for c in range(FC):
                            nc.tensor.matmul(
                                pt[:, jj, c, :],
                                xb[:, j, c * P : (c + 1) * P],
                                ident,
                                is_transpose=True,
                                start=(jj == 0 and c == 0),
                                stop=(jj == 1 and c == FC - 1),
                            )
                # PSUM -> SBUF, relayout to [p, c, j*128+r]
                xt = xt_pool.tile([P, FC, JT * P], bf16)
                for h in range(2):
                    for jj in range(2):
                        j = 2 * h + jj
                        nc.vector.tensor_copy(
                            out=xt[:, :, j * P : (j + 1) * P], in_=pts[h][:, jj]
                        )

                # H^T[m*128+p, r] over 512 rows, accumulating feature chunks;
                # then tanh(H^T + b1) -> bf16 in one op per m-chunk
                th = th_pool.tile([P, MC, JT, P], bf16)
                th_hist[g] = th
                for m in range(MC):
                    hp = hp_pool.tile([P, JT * P], f32, tag="hp")
                    for c in range(FC):
                        nc.tensor.matmul(
                            hp,
                            w1b[:, c, m, :],
                            xt[:, c, :],
                            start=(c == 0),
                            stop=(c == FC - 1),
                        )
                    nc.scalar.activation(
                        th[:, m],
                        hp.rearrange("p (j r) -> p j r", j=JT),
                        mybir.ActivationFunctionType.Tanh,
                        bias=b1s[:, m : m + 1],
                    )

                # prefetch the next group's load early
                if g + 2 < n_groups and (g + 2) not in xb_hist:
                    emit_load(g + 2)

                # pipelined: scores for g-1, bag accumulation for g-2
                if g >= 1:
                    emit_s(g - 1)
                if g >= 2:
                    emit_u(g - 2)

            # drain: scores for the last group first so its exp (ACT)
            # overlaps the U matmuls of g-2 on PE
            emit_s(n_groups - 1)
            emit_u(n_groups - 2)
            emit_u(n_groups - 1)

            # ---- epilogue: DMA results out ----
            nc.sync.dma_start(
                out=u_out.rearrange("(o b) f -> o b f", o=1), in_=u_sb
            )
            wf = out_pool.tile([P, n_tiles], f32)
            nc.vector.tensor_copy(out=wf, in_=wsave)
            nc.sync.dma_start(out=w_out[:, :], in_=wf)

    nc.compile()
    return nc


def x_ap_rearr(t, pattern, **axes):
    """rearrange a DRAM tensor handle's access pattern."""
    return t.rearrange(pattern, **axes)


def _run_device(X, W1, b1, W2, bag_rows, trace=False, trace_kwargs=None):
    from concourse.bass_utils import run_bass_kernel_spmd

    rows_per_core = X.shape[0] // N_CORES
    n_tiles = rows_per_core // P
    tiles_per_bag = bag_rows // P
    n_cols = n_tiles // tiles_per_bag
    tile_col = [t // tiles_per_bag for t in range(n_tiles)]

    key = (rows_per_core, bag_rows)
    if key in _COMPILED_CACHE:
        nc = _COMPILED_CACHE[key]
    else:
        nc = _build_program(n_tiles, tile_col, n_cols)
        _COMPILED_CACHE[key] = nc

    in_maps = []
    for c in range(N_CORES):
        in_maps.append(
            {
                "x": np.ascontiguousarray(
                    X[c * rows_per_core : (c + 1) * rows_per_core], np.float32
                ),
                "w1": np.ascontiguousarray(W1, np.float32),
                "b1": np.ascontiguousarray(b1, np.float32),
                "w2": np.ascontiguousarray(W2.reshape(HID, 1), np.float32),
            }
        )
    kw = dict(trace_kwargs or {})
    res = run_bass_kernel_spmd(
        nc, in_maps, list(range(N_CORES)), trace=trace, **kw
    )

    U = np.zeros((N_CORES * n_cols, F), np.float32)
    Z = np.float64(0.0)
    for c in range(N_CORES):
        U[c * n_cols : (c + 1) * n_cols] = res.results[c]["u"]
        Z += np.float64(res.results[c]["w"]).sum()
    return U, Z, res


def _kernel_numpy(instance_features, bag_sizes, W1, b1, W2, b2):
    """Exact-math fallback for bag layouts the device program doesn't cover."""
    X = np.asarray(instance_features, np.float32)
    s = np.tanh(X @ W1 + b1) @ W2.reshape(-1, 1) + np.asarray(b2).reshape(1, -1)
    s = s - s.max()
    w = np.exp(s)
    w = w / w.sum()
    offsets = np.cumsum(np.asarray(bag_sizes, np.int64))
    seg = np.searchsorted(offsets, np.arange(X.shape[0]), side="right")
    out = np.zeros((len(bag_sizes), X.shape[1]), np.float32)
    np.add.at(out, seg[seg < len(bag_sizes)], (X * w)[seg < len(bag_sizes)])
    return out


def kernel(**inputs):
    X = np.asarray(inputs["instance_features"], np.float32)
    bag_sizes = np.asarray(inputs["bag_sizes"], np.int64)
    W1 = np.asarray(inputs["W1"], np.float32)
    b1 = np.asarray(inputs["b1"], np.float32)
    W2 = np.asarray(inputs["W2"], np.float32)
    b2 = np.asarray(inputs["b2"], np.float32)

    T, Fdim = X.shape
    B = bag_sizes.shape[0]
    bag = int(bag_sizes[0]) if B else 0
    # Device path constraints: equal whole bags per core, 512-row groups,
    # and the row permutation needs bag_rows % 512 == 0.
    aligned = (
        Fdim == F
        and B > 0
        and np.all(bag_sizes == bag)
        and bag % 512 == 0
        and bag * B == T
        and T % N_CORES == 0
        and (T // N_CORES) % (4 * P) == 0
        and (T // N_CORES) % bag == 0
    )
    if not aligned:
        return _kernel_numpy(X, bag_sizes, W1, b1, W2, b2)

    U, Z, _ = _run_device(X, W1, b1, W2, bag)
    return (U / np.float32(Z)).astype(np.float32)


# revision 27
# speedup vs baseline: 1.0962x; 1.0962x over previous
"""AttentionMILPooling Trainium2 kernel.

Math (matches the jax reference):
    scores  = tanh(X @ W1 + b1) @ W2 + b2          # [T, 1]
    weights = softmax(scores, axis=0)              # global over all T
    out[b]  = sum_{i in bag b} weights[i] * X[i]   # [64, 512]

Key identities used:
  * b2 cancels exactly in the softmax, so it is dropped.
  * scores are bounded (|s| <= sum|W2| ~ 13) so no max-subtraction is
    needed; exp stays well inside fp32 range.
  * out[b] = (sum_{i in b} exp(s_i) * X_i) / Z with Z = sum_i exp(s_i):
    each core computes unnormalized per-bag sums U and returns all its
    per-row exp values; the host sums Z globally and divides once.
  * All math is row-order-free within a bag, so rows are permuted by the
    DMA layout (each partition loads 2 consecutive DRAM rows -> 4KB
    descriptors) without any correction.

Sharding: instances split contiguously across 8 cores (16384 rows each);
with equal 2048-row bags every core owns 8 whole bags. The tiny MLP
weights are replicated. Each core returns U [8 x 512] and its exp
values w [128 x 128]; the host concatenates and divides by global Z.

Per-core device pipeline, per 512-row group (32 groups), all matmul
operands bf16 with fp32 PSUM accumulation:
  SWDGE : X fp32 -> SBUF bf16 (cast in DMA)             xb [128,4,512]
  PE    : 16x transpose -> X^T chunks in PSUM
  DVE   : 4x copy PSUM->SBUF                            xt [128,4,512]
  PE    : 8x matmul H^T = W1^T @ X^T (accum f chunks)   [128,512] f32
  ACT   : 2x tanh(H^T + b1) -> bf16 (b1 as per-partition bias)
  PE    : 8x matmul s = tanhH^T.T @ W2 (accum h chunks) [128,4] f32
  ACT   : 1x exp(s) -> persistent wsave columns (bf16)
  PE    : 4x matmul U[bag] += w_tile^T @ X_tile (w stationary, N=512)
Emission is software-pipelined (s for group g-1, U for g-2) so the
in-order PE stream never waits on same-group ACT results. Per-bag U
accumulates in a dedicated [1,512] PSUM bank over the bag's 16 tiles
(PSUM start=True zeroes a whole 2KB region, so each accumulation group
owns its bank), then is copied to SBUF and DMA'd out.
"""

import numpy as np

N_CORES = 8
T_FULL = 131072
F = 512  # feature dim
HID = 256  # hidden dim
B_FULL = 64  # number of bags
P = 128  # partitions

_COMPILED_CACHE = {}


def _build_program(n_tiles, tile_col, n_cols):
    """Build the SPMD bass program.

    n_tiles: number of 128-row tiles per core (must be divisible by 4).
    tile_col: list, local bag-column index for each tile (same on all cores).
    n_cols: number of local bag columns.
    """
    import concourse.bacc as bacc
    import concourse.mybir as mybir
    from concourse.tile import TileContext
    from concourse.masks import make_identity

    f32 = mybir.dt.float32
    bf16 = mybir.dt.bfloat16
    FC = F // P  # 4 feature chunks
    MC = HID // P  # 2 hidden chunks
    rows_per_core = n_tiles * P
    JT = 4  # 128-row subtiles per group
    GR = JT * P  # rows per group
    n_groups = n_tiles // JT

    nc = bacc.Bacc(
        "TRN2", target_bir_lowering=False, debug=False, num_devices=N_CORES
    )

    x = nc.declare_dram_parameter("x", [rows_per_core, F], f32, isOutput=False)
    w1 = nc.declare_dram_parameter("w1", [F, HID], f32, isOutput=False)
    b1 = nc.declare_dram_parameter("b1", [HID], f32, isOutput=False)
    w2 = nc.declare_dram_parameter("w2", [HID, 1], f32, isOutput=False)
    u_out = nc.declare_dram_parameter("u", [n_cols, F], f32, isOutput=True)
    w_out = nc.declare_dram_parameter("w", [P, n_tiles], f32, isOutput=True)

    with TileContext(nc) as tc:
        with (
            tc.tile_pool(name="const", bufs=1) as const_pool,
            tc.tile_pool(name="xb", bufs=8) as xb_pool,
            tc.tile_pool(name="xt", bufs=4) as xt_pool,
            tc.tile_pool(name="th", bufs=3) as th_pool,
            tc.tile_pool(name="out_sb", bufs=1) as out_pool,
            tc.tile_pool(name="pt", bufs=4, space="PSUM") as pt_pool,
            tc.tile_pool(name="hp", bufs=2, space="PSUM") as hp_pool,
            tc.tile_pool(name="sp", bufs=1, space="PSUM") as sp_pool,
            tc.tile_pool(name="acc", bufs=1, space="PSUM") as acc_pool,
        ):
            # ---- prefetch the first X groups before the const DMAs so the
            # transposes (which need only xb + ident) start ASAP ----
            xb_hist = {}

            def emit_load(gg):
                # load + cast fp32 -> bf16 during DMA (SWDGE). Rows are
                # permuted so each partition reads 4 consecutive DRAM rows
                # (8KB contiguous, one descriptor run per partition):
                # subtile j holds rows {4p + j} of the 512-row group. All
                # downstream math is row-order-free within a bag (sums),
                # and every 512-row group lies in one bag (bag % 512 == 0).
                xb = xb_pool.tile([P, JT, F], bf16, name="xb", tag="xb")
                xb_hist[gg] = xb
                x_src = x[gg * GR : (gg + 1) * GR, :].rearrange(
                    "(p q) f -> p q f", p=P
                )
                nc.gpsimd.dma_start(out=xb, in_=x_src)

            def emit_load_split(gg):
                # first groups: two half-size DMAs so the earliest
                # transposes can start sooner during pipeline ramp-up.
                # Permutation differs (rows {2p+q} per 256-row window) but
                # any within-bag permutation is legal.
                xb = xb_pool.tile([P, JT, F], bf16, name="xb", tag="xb")
                xb_hist[gg] = xb
                for h in range(2):
                    sl = x[
                        gg * GR + h * (GR // 2) : gg * GR + (h + 1) * (GR // 2), :
                    ].rearrange("(p q) f -> p q f", p=P)
                    nc.gpsimd.dma_start(out=xb[:, 2 * h : 2 * h + 2, :], in_=sl)

            emit_load_split(0)
            if n_groups > 1:
                emit_load_split(1)

            # ---- constants ----
            ident = const_pool.tile([P, P], bf16)
            make_identity(nc, ident)

            # W1 -> bf16 chunks: w1b[p, c, m, j] = W1[c*128+p, m*128+j]
            w1f = const_pool.tile([P, FC, MC, P], f32)
            nc.sync.dma_start(
                out=w1f, in_=w1.rearrange("(c p) (m j) -> p c m j", p=P, j=P)
            )
            w1b = const_pool.tile([P, FC, MC, P], bf16)
            nc.vector.tensor_copy(out=w1b, in_=w1f)

            # W2 -> bf16 chunks: w2b[p, m, 0] = W2[m*128+p, 0]
            w2f = const_pool.tile([P, MC, 1], f32)
            nc.sync.dma_start(
                out=w2f, in_=w2.rearrange("(m p) one -> p m one", p=P)
            )
            w2b = const_pool.tile([P, MC, 1], bf16)
            nc.vector.tensor_copy(out=w2b, in_=w2f)

            # b1 per-partition bias: b1s[p, m] = b1[m*128+p]
            b1s = const_pool.tile([P, MC], f32)
            nc.sync.dma_start(out=b1s, in_=b1.rearrange("(m p) -> p m", p=P))

            # softmax weights for every row, bf16 (also read back by the host
            # to form the global softmax denominator)
            wsave = const_pool.tile([P, n_tiles], bf16)

            # ---- per-bag accumulator handling ----
            # matmul outputs must start at partition 0/32/64, so each bag
            # accumulates in its own [1, F] PSUM tile (one bank) for the
            # contiguous run of its tiles, then is copied into its row of
            # the SBUF result before the tile is recycled.
            u_sb = out_pool.tile([1, n_cols, F], f32)
            first_tile = {}
            last_tile = {}
            for t, cl in enumerate(tile_col):
                first_tile.setdefault(cl, t)
                last_tile[cl] = t
            u_bag = [None] * n_cols

            # ---- main loop over 512-row groups (software-pipelined) ----
            # PE executes in emission order, so same-group dependencies on
            # ACT (tanh -> s, exp -> U) would stall it. Emit s for group
            # g-1 and U for group g-2: their ACT inputs are long ready.
            th_hist = {}
            sp_hist = {}

            def emit_s(gg):
                th_g = th_hist[gg]
                sp = sp_pool.tile([P, JT], f32, name="sp", tag="sp")
                sp_hist[gg] = sp
                for j in range(JT):
                    for m in range(MC):
                        nc.tensor.matmul(
                            sp[:, j : j + 1],
                            th_g[:, m, j, :],
                            w2b[:, m, :],
                            start=(j == 0 and m == 0),
                            stop=(j == JT - 1 and m == MC - 1),
                        )
                nc.scalar.activation(
                    wsave[:, gg * JT : (gg + 1) * JT],
                    sp,
                    mybir.ActivationFunctionType.Exp,
                )
                del sp_hist[gg]
                del th_hist[gg]

            def emit_u(gg):
                xb_g = xb_hist.pop(gg)
                for j in range(JT):
                    t = JT * gg + j
                    col = tile_col[t]
                    if u_bag[col] is None:
                        u_bag[col] = acc_pool.tile(
                            [1, F], f32, name="u_bag", tag="u_bag"
                        )
                    nc.tensor.matmul(
                        u_bag[col],
                        wsave[:, t : t + 1],
                        xb_g[:, j, :],
                        start=(t == first_tile[col]),
                        stop=(t == last_tile[col]),
                    )
                    if t == last_tile[col]:
                        nc.vector.tensor_copy(
                            out=u_sb[:, col, :], in_=u_bag[col]
                        )
                        u_bag[col] = None

            for g in range(n_groups):
                if g not in xb_hist:
                    emit_load(g)
                xb = xb_hist[g]

                # transpose all subtiles/chunks: two PSUM tiles of 8 each
                pts = []
                for h in range(2):
                    pt = pt_pool.tile([P, 2, FC, P], bf16, tag="pt")
                    pts.append(pt)
                    for jj in range(2):
                        j = 2 * h + jj
                        for c in range(FC):
                            nc.tensor.matmul(
                                pt[:, jj, c, :],
                                xb[:, j, c * P : (c + 1) * P],
                                ident,
                                is_transpose=True,
                                start=(jj == 0 and c == 0),
                                stop=(jj == 1 and c == FC - 1),
                            )
                # PSUM -> SBUF, relayout to [p, c, j*128+r]
                xt = xt_pool.tile([P, FC, JT * P], bf16)
                for h in range(2):
                    for jj in range(2):
                        j = 2 * h + jj
                        nc.vector.tensor_copy(
                            out=xt[:, :, j * P : (j + 1) * P], in_=pts[h][:, jj]
                        )

                # H^T[m*128+p, r] over 512 rows, accumulating feature chunks;
                # then tanh(H^T + b1) -> bf16 in one op per m-chunk
                th = th_pool.tile([P, MC, JT, P], bf16)
                th_hist[g] = th
                for m in range(MC):
                    hp = hp_pool.tile([P, JT * P], f32, tag="hp")
                    for c in range(FC):
                        nc.tensor.matmul(
                            hp,
                            w1b[:, c, m, :],
                            xt[:, c, :],
                            start=(c == 0),
                            stop=(c == FC - 1),
                        )
                    nc.scalar.activation(
                        th[:, m],
                        hp.rearrange("p (j r) -> p j r", j=JT),
                        mybir.ActivationFunctionType.Tanh,
                        bias=b1s[:, m : m + 1],
                    )

                # prefetch the next group's load early
                if g + 2 < n_groups and (g + 2) not in xb_hist:
                    emit_load(g + 2)

                # pipelined: scores for g-1, bag accumulation for g-2
                if g >= 1:
                    emit_s(g - 1)
                if g >= 2:
                    emit_u(g - 2)

            # drain: scores for the last group first so its exp (ACT)
            # overlaps the U matmuls of g-2 on PE
            emit_s(n_groups - 1)
            emit_u(n_groups - 2)
            emit_u(n_groups - 1)

            # ---- epilogue: DMA results out ----
            nc.sync.dma_start(
                out=u_out.rearrange("(o b) f -> o b f", o=1), in_=u_sb
            )
            wf = out_pool.tile([P, n_tiles], f32)
            nc.vector.tensor_copy(out=wf, in_=wsave)
            nc.sync.dma_start(out=w_out[:, :], in_=wf)

    nc.compile()
    return nc


def x_ap_rearr(t, pattern, **axes):
    """rearrange a DRAM tensor handle's access pattern."""
    return t.rearrange(pattern, **axes)


def _run_device(X, W1, b1, W2, bag_rows, trace=False, trace_kwargs=None):
    from concourse.bass_utils import run_bass_kernel_spmd

    rows_per_core = X.shape[0] // N_CORES
    n_tiles = rows_per_core // P
    tiles_per_bag = bag_rows // P
    n_cols = n_tiles // tiles_per_bag
    tile_col = [t // tiles_per_bag for t in range(n_tiles)]

    key = (rows_per_core, bag_rows)
    if key in _COMPILED_CACHE:
        nc = _COMPILED_CACHE[key]
    else:
        nc = _build_program(n_tiles, tile_col, n_cols)
        _COMPILED_CACHE[key] = nc

    in_maps = []
    for c in range(N_CORES):
        in_maps.append(
            {
                "x": np.ascontiguousarray(
                    X[c * rows_per_core : (c + 1) * rows_per_core], np.float32
                ),
                "w1": np.ascontiguousarray(W1, np.float32),
                "b1": np.ascontiguousarray(b1, np.float32),
                "w2": np.ascontiguousarray(W2.reshape(HID, 1), np.float32),
            }
        )
    kw = dict(trace_kwargs or {})
    res = run_bass_kernel_spmd(
        nc, in_maps, list(range(N_CORES)), trace=trace, **kw
    )

    U = np.zeros((N_CORES * n_cols, F), np.float32)
    Z = np.float64(0.0)
    for c in range(N_CORES):
        U[c * n_cols : (c + 1) * n_cols] = res.results[c]["u"]
        Z += np.float64(res.results[c]["w"]).sum()
    return U, Z, res


def _kernel_numpy(instance_features, bag_sizes, W1, b1, W2, b2):
    """Exact-math fallback for bag layouts the device program doesn't cover."""
    X = np.asarray(instance_features, np.float32)
    s = np.tanh(X @ W1 + b1) @ W2.reshape(-1, 1) + np.asarray(b2).reshape(1, -1)
    s = s - s.max()
    w = np.exp(s)
    w = w / w.sum()
    offsets = np.cumsum(np.asarray(bag_sizes, np.int64))
    seg = np.searchsorted(offsets, np.arange(X.shape[0]), side="right")
    out = np.zeros((len(bag_sizes), X.shape[1]), np.float32)
    np.add.at(out, seg[seg < len(bag_sizes)], (X * w)[seg < len(bag_sizes)])
    return out


def kernel(**inputs):
    X = np.asarray(inputs["instance_features"], np.float32)
    bag_sizes = np.asarray(inputs["bag_sizes"], np.int64)
    W1 = np.asarray(inputs["W1"], np.float32)
    b1 = np.asarray(inputs["b1"], np.float32)
    W2 = np.asarray(inputs["W2"], np.float32)
    b2 = np.asarray(inputs["b2"], np.float32)

    T, Fdim = X.shape
    B = bag_sizes.shape[0]
    bag = int(bag_sizes[0]) if B else 0
    # Device path constraints: equal whole bags per core, 512-row groups,
    # and the row permutation needs bag_rows % 512 == 0.
    aligned = (
        Fdim == F
        and B > 0
        and np.all(bag_sizes == bag)
        and bag % 512 == 0
        and bag * B == T
        and T % N_CORES == 0
        and (T // N_CORES) % (4 * P) == 0
        and (T // N_CORES) % bag == 0
    )
    if not aligned:
        return _kernel_numpy(X, bag_sizes, W1, b1, W2, b2)

    U, Z, _ = _run_device(X, W1, b1, W2, bag)
    return (U / np.float32(Z)).astype(np.float32)
